# revision 32
# baseline (speedup 1.0000x reference)
"""Trainium2 Bass kernel for NeuroVPR Vanilla SNN (3-layer LIF, T=3).

Data-parallel over batch: B=16384 -> 2048 per core x 8 cores.

Math (per timestep, per layer): v = (v_prev + h)/2; s = (v>=1); v *= (1-s).
The LIF recurrence is homogeneous and the decay is a power of 2, so each
layer runs in a scaled basis u_t = 2^t * 2c * v_t (c = weight prescale,
32 for L1 / 16 for L2-L3, lifting weights out of fp8's subnormal range):
    u_t = m_{t-1} + 2^t * psum_t      (the *0.5 decay cancels)
    s_t = (u_t >= 2^t * 2c)
    m_t = u_t * (u_t < 2^t * 2c)
The 2^t factor rides the ScalarE extract's free `scale` field; thresholds
double each timestep (exact powers of 2). Spike decisions match the fp32
recurrence up to matmul quantization error.

All matmuls are fp8e4 perf_mode=DoubleRow (K=256/instr, warm issue rate
216 ns at N=512 - measured 2x over bf16). Hidden-layer spikes live in a
+/-1 (ScalarE Sign, t=1) or +/-0.5 (VectorE is_ge/sub, t=0 and t=2)
basis: the next layer's ScalarE extract scale absorbs the basis change
(SSC) and the row-sum correction rides that layer's bias column, both
precomputed on host from the quantized weights. L1's bias rides a ones
row appended to x (pad row D) so its beta columns are zero.

Engine split per [128,1024] psum span (2 banks, measured costs):
  ScalarE: hb = Ident(SSC*psum + beta_col) -> bf16   (~1.09us)
  VectorE: u = m + hb          (tt add bf16, ~0.69us; t=0: u = hb)
  spike:   ScalarE Sign at t=1 (DVE-heavy step), VectorE ts otherwise
  VectorE: m = (u < th)*u      (stt bf16, ~1.22us; skipped at t=T-1)
At t=T-1, L1/L3 chains skip the extract and run on VectorE straight from
PSUM (u = dmsc*m + psum) so ScalarE never serializes the endgame; L3's
row-sum correction is then added into PSUM by a K=1 ones-matmul. All
t=T-1 chains process per-512 chunks (tile deps are range-based, so each
downstream matmul starts on the first half-span) and the final output
DMAs alternate sync/scalar queues.
GpSimd is avoided entirely (measured ~8us per [128,512] op).

Schedule: per timestep L1 runs as two half-batch passes (2 double-bank
psum groups each, k inner). L2(t-1)/L3(t-1) matmul groups are hooked
into the k-loops of later passes so the in-order PE queue never waits on
their PSUM-extract dependencies; only L2(T-1)+L3(T-1) trail the last
pass. x half-tiles ([128, 2048] fp8 per k-slab per half-batch) are
DMA-prefetched one L1 pass ahead.
"""
import os
import numpy as np
import ml_dtypes

B, T, D = 16384, 3, 2752
DP = 2816          # D padded to 11*256
KD = DP // 256     # 11 DoubleRow contraction slabs
H, O = 256, 100
OP = 112           # O padded so the DoubleRow pair-stride is 16B-aligned
NCORES = 8
BC = B // NCORES   # 2048
NB = 512           # matmul free-dim block
WB = 1024          # LIF elementwise span (2 psum banks)
HB = BC // 2       # half-batch per L1 pass (1024)

SC1, SC2 = 32.0, 8.0    # weight prescale: L1; L2/L3 (+/-1 spike basis)
TH1, TH2 = 64.0, 32.0   # base thresholds (scaled x2 each timestep)
EPS = 0.0625            # tie-break so Sign(u - (th-EPS)) == +/-1 with s=1 at u==th

_compiled = None
last_results = None  # BassKernelResults of the most recent run (for profiling)


def _build():
    from contextlib import ExitStack
    import concourse.bass as bass
    import concourse.mybir as mybir
    import concourse.tile as tile
    from concourse import bacc

    f8 = mybir.dt.float8e4
    bf16 = mybir.dt.bfloat16
    f32 = mybir.dt.float32
    A = mybir.AluOpType
    DR = mybir.MatmulPerfMode.DoubleRow
    IDENT = mybir.ActivationFunctionType.Identity
    SIGN = mybir.ActivationFunctionType.Sign

    nc = bacc.Bacc("TRN2", target_bir_lowering=False, debug=False)
    x = nc.dram_tensor("x", [T, KD, 2, 128, HB * 2], f8, kind="ExternalInput").ap()
    w1 = nc.dram_tensor("w1", [128, KD * 2 * H], f8, kind="ExternalInput").ap()
    w2 = nc.dram_tensor("w2", [128, 2 * H], f8, kind="ExternalInput").ap()
    w3 = nc.dram_tensor("w3", [128, 2 * OP], f8, kind="ExternalInput").ap()
    bias = nc.dram_tensor("bias", [128, 21], f32, kind="ExternalInput").ap()
    brow = nc.dram_tensor("brow", [1, 512], mybir.dt.bfloat16,
                          kind="ExternalInput").ap()
    cw = nc.dram_tensor("cw", [128, 4 * 256], f8, kind="ExternalInput").ap()
    out = nc.dram_tensor("out", [O, BC], f32, kind="ExternalOutput").ap()

    with tile.TileContext(nc) as tc, ExitStack() as ctx:
        wp = ctx.enter_context(tc.tile_pool(name="wp", bufs=1))
        xp = ctx.enter_context(tc.tile_pool(name="xp", bufs=24))
        pp1 = ctx.enter_context(tc.tile_pool(name="pp1", bufs=3, space="PSUM"))
        pp23 = ctx.enter_context(tc.tile_pool(name="pp23", bufs=1, space="PSUM"))
        sp = ctx.enter_context(tc.tile_pool(name="sp", bufs=1))
        tp = ctx.enter_context(tc.tile_pool(name="tp", bufs=6))

        # ---- weights / bias loads (k=0 slab first, on its own queue) ----
        w1t = wp.tile([128, KD * 2 * H], f8)
        nc.gpsimd.dma_start(out=w1t[:, 0:512], in_=w1[:, 0:512])
        for c0, c1 in ((512, 2048), (2048, 4096), (4096, KD * 512)):
            nc.scalar.dma_start(out=w1t[:, c0:c1], in_=w1[:, c0:c1])
        w1o = w1t[:, :].rearrange("p (k two m) -> p k two m", k=KD, two=2)
        w2t = wp.tile([128, 2 * H], f8)
        nc.gpsimd.dma_start(out=w2t[:, :], in_=w2[:, :])
        w2o = w2t[:, :].rearrange("p (two m) -> p two m", two=2)
        w3t = wp.tile([128, 2 * OP], f8)
        nc.gpsimd.dma_start(out=w3t[:, :], in_=w3[:, :])
        w3o = w3t[:, :].rearrange("p (two m) -> p two m", two=2)
        bt = wp.tile([128, 21], f32)
        nc.gpsimd.dma_start(out=bt[:, :], in_=bias[:, :])
        browt = wp.tile([1, 512], bf16)
        nc.gpsimd.dma_start(out=browt[:, :], in_=brow[:, :])
        cwt = wp.tile([128, 4 * 256], f8)
        nc.gpsimd.dma_start(out=cwt[:, :], in_=cw[:, :])
        # cwo[t-1][h]: -th1(t-1)/2^t * identity on pair-half h (L1 min-reset
        # reconstruction: psum1(t) += C * s1'(t-1))
        cwr = cwt[:, :].rearrange("p (i two m) -> p i two m", i=4, two=2)
        cwo = [[cwr[:, 2 * i + h, :, :] for h in range(2)] for i in range(2)]
        ones = wp.tile([1, NB], bf16)
        nc.vector.memset(ones[:, :], 1.0)
        # column layout (host fills): 0-5 beta1[t,h]; 6-11 beta2[t,h];
        # 12-14 beta3[t]; 15-17 -(2^t*TH1-EPS); 18-20 -(2^t*TH2-EPS)
        B1 = lambda t, h: bt[:, 2 * t + h: 2 * t + h + 1]
        B2 = lambda t, h: bt[:, 6 + 2 * t + h: 6 + 2 * t + h + 1]
        B3 = lambda t: bt[:, 12 + t: 13 + t]
        N1 = lambda t: bt[:, 15 + t: 16 + t]
        N2 = lambda t: bt[:, 18 + t: 19 + t]

        # ACT warmup: pull the activation table load off the critical path
        wu = wp.tile([128, 8], bf16)
        nc.vector.memset(wu[:, :], 0.0)
        nc.scalar.activation(wu[:, 0:4], wu[:, 4:8], IDENT, bias=bt[:, 0:1])

        # ---- persistent state (m = scaled membrane, written at t=0) ----
        m1 = [sp.tile([128, BC], bf16, tag=f"m1_{h}", name=f"m1_{h}")
              for h in range(2)]
        m2 = [sp.tile([128, BC], bf16, tag=f"m2_{h}", name=f"m2_{h}")
              for h in range(2)]
        m3 = sp.tile([128, BC], bf16, tag="m3")
        s1 = sp.tile([128, 2 * BC], f8, tag="s1")
        s2 = sp.tile([128, 2 * BC], f8, tag="s2")
        s1r = s1[:, :].rearrange("p (two n) -> p two n", two=2)
        s2r = s2[:, :].rearrange("p (two n) -> p two n", two=2)
        outsb = sp.tile([128, BC], f32, tag="outsb")

        xt = {}  # (t, k, half) -> x tile handle [128, 2*HB]

        def x_fetch(t, k, half, split=False):
            xt[t, k, half] = xp.tile([128, 2 * HB], f8, tag="x",
                                     name=f"x_{t}_{k}_{half}")
            if split:
                nc.sync.dma_start(out=xt[t, k, half][:, 0:HB],
                                  in_=x[t, k, half, :, 0:HB])
                nc.gpsimd.dma_start(out=xt[t, k, half][:, HB:],
                                    in_=x[t, k, half, :, HB:])
            else:
                nc.sync.dma_start(out=xt[t, k, half][:, :],
                                  in_=x[t, k, half, :, :])

        SSC = [2.0, 2.0, 8.0]  # 2^t x (2 if spikes were +/-0.5 basis)

        def lif(ps, m_ap, s_ap, bcol, nthcol, th, t, P=128, out_f32=False,
                sc=None, dmsc=0.25, dth=None, pd=True, mn=False):
            """Scaled-LIF on one [P, WB] psum span. At t == T-1 the whole
            chain runs on VectorE straight from PSUM (dmsc*m + psum, spike at
            dth) so ScalarE never serializes the endgame; earlier timesteps
            use the ScalarE extract + bf16 VectorE ops."""
            if t == T - 1 and pd:
                u = tp.tile([128, WB], bf16, tag="u", name="u")[:P, :]
                for c in range(2):
                    cs = slice(c * NB, (c + 1) * NB)
                    nc.vector.scalar_tensor_tensor(u[:, cs], m_ap[:, cs], dmsc,
                                                   ps[:, cs], A.mult, A.add)
                    if out_f32:
                        nc.vector.tensor_scalar(s_ap[:, cs], u[:, cs], dth,
                                                None, A.is_ge)
                    else:
                        nc.vector.tensor_scalar(s_ap[:, cs], u[:, cs], dth,
                                                0.5, A.is_ge, A.subtract)
                return
            if t == T - 1:
                # PATH-A endgame (L2): per-512 chunks so downstream range
                # consumers start on the first half
                hb = tp.tile([128, WB], bf16, tag="hb", name="hb")[:P, :]
                u = tp.tile([128, WB], bf16, tag="u", name="u")[:P, :]
                for c in range(2):
                    cs = slice(c * NB, (c + 1) * NB)
                    nc.scalar.activation(hb[:, cs], ps[:, cs], IDENT,
                                         bias=bcol[:P, :], scale=sc)
                    nc.vector.tensor_tensor(u[:, cs], m_ap[:, cs], hb[:, cs],
                                            A.add)
                    nc.vector.tensor_scalar(s_ap[:, cs], u[:, cs],
                                            th * 2 ** t, 0.5,
                                            A.is_ge, A.subtract)
                return
            hb = tp.tile([128, WB], bf16, tag="hb", name="hb")[:P, :]
            nc.scalar.activation(hb, ps, IDENT, bias=bcol[:P, :],
                                 scale=float(2 ** t) if sc is None else sc)
            if t == 0:
                u = hb
            else:
                u = tp.tile([128, WB], bf16, tag="u", name="u")[:P, :]
                nc.vector.tensor_tensor(u, m_ap, hb, A.add)
            if s_ap is not None:
                if t == 1:
                    nc.scalar.activation(s_ap, u, SIGN, bias=nthcol[:P, :])
                else:
                    nc.vector.tensor_scalar(s_ap, u, th * 2 ** t, 0.5,
                                            A.is_ge, A.subtract)
            if mn:
                nc.vector.tensor_scalar(m_ap, u, th * 2 ** t, None, A.min)
            else:
                nc.vector.scalar_tensor_tensor(m_ap, u, th * 2 ** t, u,
                                               A.is_lt, A.mult)

        def l1_pass(t, half, hooks=None):
            """One half-batch L1 pass: 2 double-bank psum groups, k inner.
            Prefetches the next pass's x tiles; `hooks[k]` emits interleaved
            L2/L3 work (their MMs slot into the PE stream between slabs)."""
            boff = half * HB
            ps = [pp1.tile([128, WB], f32, tag="ps1", name=f"ps1_{t}_{half}_{h}")
                  for h in range(2)]
            for k in range(KD):
                for fn in (hooks or {}).get(k, []):
                    fn()
                xr = xt[t, k, half][:, :].rearrange("p (two n) -> p two n", two=2)
                for h in range(2):
                    for b in range(2):
                        nc.tensor.matmul(
                            ps[h][:, b * NB:(b + 1) * NB],
                            w1o[:, k, :, h * 128:(h + 1) * 128],
                            xr[:, :, b * NB:(b + 1) * NB],
                            start=(k == 0), stop=(k == KD - 1), perf_mode=DR,
                            skip_group_check=True)
                        if k == 0 and t >= 1:
                            nc.tensor.matmul(
                                ps[h][:, b * NB:(b + 1) * NB],
                                cwo[t - 1][h],
                                s1r[:, :, boff + b * NB: boff + (b + 1) * NB],
                                start=False, stop=False, perf_mode=DR,
                                skip_group_check=True)
                if half == 0:
                    x_fetch(t, k, 1)
                elif t < T - 1:
                    x_fetch(t + 1, k, 0)
            for h in range(2):
                bs = slice(boff, boff + WB)
                lif(ps[h][:, :], m1[h][:, bs],
                    s1[:, h * BC + boff: h * BC + boff + WB],
                    B1(t, h), N1(t), TH1, t, dmsc=0.25, dth=TH1, mn=True)

        def l2_group(t, h, bp, pool, tag):
            ps2 = pool.tile([128, WB], f32, tag=tag, name=f"ps2_{t}_{h}_{bp}")
            for b in range(2):
                nc.tensor.matmul(ps2[:, b * NB:(b + 1) * NB],
                                 w2o[:, :, h * 128:(h + 1) * 128],
                                 s1r[:, :, (2 * bp + b) * NB:(2 * bp + b + 1) * NB],
                                 start=True, stop=True, perf_mode=DR,
                                 skip_group_check=True)
            return ps2

        def l3_group(t, bp, pool, tag):
            ps3 = pool.tile([128, WB], f32, tag=tag, name=f"ps3_{t}_{bp}")
            corr = (t == T - 1)
            for b in range(2):
                nc.tensor.matmul(ps3[:OP, b * NB:(b + 1) * NB], w3o[:, :, :],
                                 s2r[:, :, (2 * bp + b) * NB:(2 * bp + b + 1) * NB],
                                 start=True, stop=not corr, perf_mode=DR,
                                 skip_group_check=True)
                if corr:
                    nc.tensor.matmul(ps3[:OP, b * NB:(b + 1) * NB],
                                     browt[0:1, 256:256 + OP],
                                     ones[0:1, :], start=False, stop=True,
                                     skip_group_check=True)
            return ps3

        def l2_one(t, h, bp, pool=None, tag=None):
            bs = slice(bp * WB, (bp + 1) * WB)
            ps2 = l2_group(t, h, bp, pool or pp23, tag or "ps23")
            lif(ps2[:, :], m2[h][:, bs],
                s2[:, h * BC + bp * WB: h * BC + (bp + 1) * WB],
                B2(t, h), N2(t), TH2, t, sc=SSC[t], pd=False)

        def l3_one(t, bp, pool=None, tag=None):
            bs = slice(bp * WB, (bp + 1) * WB)
            ps3 = l3_group(t, bp, pool or pp23, tag or "ps23")
            if t != T - 1:
                lif(ps3[:OP, :], m3[:OP, bs], None, B3(t), None, TH2, t,
                    P=OP, sc=SSC[t])
            else:
                lif(ps3[:OP, :], m3[:OP, bs], outsb[:OP, bs], B3(t), None,
                    TH2, t, P=OP, out_f32=True, sc=SSC[t], dmsc=0.125,
                    dth=16.0)
                for c in range(2):
                    cs = slice(bp * WB + c * NB, bp * WB + (c + 1) * NB)
                    (nc.sync if c == 0 else nc.scalar).dma_start(
                        out=out[:, cs], in_=outsb[:O, cs])

        for k in range(KD):
            x_fetch(0, k, 0)
        l1_pass(0, 0)
        l1_pass(0, 1)
        l1_pass(1, 0)
        l1_pass(1, 1, hooks={3: [lambda: l2_one(0, 0, 0)],
                             5: [lambda: l2_one(0, 0, 1)],
                             7: [lambda: l2_one(0, 1, 0)],
                             9: [lambda: l2_one(0, 1, 1)]})
        l1_pass(2, 0, hooks={1: [lambda: l2_one(1, 0, 0)],
                             3: [lambda: l2_one(1, 0, 1)],
                             5: [lambda: l2_one(1, 1, 0)],
                             7: [lambda: l2_one(1, 1, 1)],
                             9: [lambda: l3_one(0, 0)]})
        l1_pass(2, 1, hooks={2: [lambda: l3_one(0, 1)],
                             6: [lambda: l3_one(1, 0)],
                             9: [lambda: l3_one(1, 1)]})
        # endgame: only L2(T-1) (ScalarE extract) + L3(T-1) (VectorE direct)
        t_ = T - 1
        l2_one(t_, 0, 0, pp1, "ps1")
        l2_one(t_, 1, 0, pp1, "ps1")
        l2_one(t_, 0, 1, pp1, "ps1")
        l2_one(t_, 1, 1, pp1, "ps1")
        l3_one(t_, 0)
        l3_one(t_, 1)

    nc.compile()
    return nc


def kernel(dvs, W1, b1, W2, b2, W3, b3):
    global _compiled, last_results
    from concourse.bass_utils import run_bass_kernel_spmd

    if _compiled is None:
        _compiled = _build()
    nc = _compiled

    f8 = ml_dtypes.float8_e4m3

    def q8(a, scale):
        return np.clip(a * scale, -240.0, 240.0).astype(f8)

    # x: [B, T, D] -> fp8 [T, KD, 128, 2, B]  (d = k*256 + two*128 + p)
    x8 = q8(dvs, 1.0).transpose(1, 2, 0)          # [T, D, B]
    X = np.zeros((T, KD, 2, 128, B), dtype=f8)
    X.reshape(T, DP, B)[:, :D, :] = x8
    X.reshape(T, DP, B)[:, D, :] = f8(1.0)        # bias row (w1 row D = c1*b1)
    X.reshape(T, DP, B)[1:, D + 1, :] = f8(1.0)   # min-reset const row (t>=1)
    X = np.ascontiguousarray(X.transpose(0, 1, 3, 2, 4))  # [T, KD, 128, 2, B]

    # w1: [DP, H] scaled by SC1 -> [128, KD, 2, H]
    w1p = np.zeros((KD, 2, 128, H), dtype=f8)
    w1p.reshape(DP, H)[:D, :] = q8(W1.T, SC1)
    w1p.reshape(DP, H)[D, :] = q8(b1, SC1)
    w1p.reshape(DP, H)[D + 1, :] = f8(-16.0)      # min-reset constant
    w1p = np.ascontiguousarray(w1p.transpose(2, 0, 1, 3)).reshape(128, KD * 2 * H)
    # w2/w3 scaled by SC2 (+/-1 spike basis)
    w2q = q8(W2.T, SC2)                            # [H, H] j-major
    w2p = np.ascontiguousarray(
        w2q.reshape(2, 128, H).transpose(1, 0, 2)).reshape(128, 2 * H)
    w3q = np.zeros((H, OP), dtype=f8)
    w3q[:, :O] = q8(W3.T, SC2)
    w3p = np.ascontiguousarray(
        w3q.reshape(2, 128, OP).transpose(1, 0, 2)).reshape(128, 2 * OP)

    # bias/threshold columns; row-sum corrections use the quantized weights
    rs2 = w2q.astype(np.float64).sum(axis=0)       # [H]
    rs3 = w3q.astype(np.float64).sum(axis=0)       # [OP]
    bc = np.zeros((128, 21), dtype=np.float32)
    for t in range(T):
        p2 = float(2 ** t)
        for h in range(2):
            bc[:, 6 + 2 * t + h] = p2 * (rs2[h * 128:(h + 1) * 128]
                                         + 2 * SC2 * b2[h * 128:(h + 1) * 128])
        bc[:OP, 12 + t] = p2 * rs3
        bc[:O, 12 + t] += p2 * 2 * SC2 * b3
        bc[:, 15 + t] = -(p2 * TH1 - EPS)
        bc[:, 18 + t] = -(p2 * TH2 - EPS)

    # L1 min-reset correction diagonals: psum1(t) += C_t * s1'(t-1)
    cwh = np.zeros((128, 4, 2, 128), dtype=f8)
    for i, C in enumerate((-32.0, -16.0)):
        for h in range(2):
            for q in range(128):
                cwh[q, 2 * i + h, h, q] = C
    cwh = cwh.reshape(128, 4 * 256)

    br = np.zeros((1, 512), dtype=np.float32)
    br[0, :H] = (rs2 + 2 * SC2 * b2) / 2
    br[0, H:H + OP] = rs3 / 2
    br[0, H:H + O] += SC2 * b3
    br = br.astype(ml_dtypes.bfloat16)

    in_maps = []
    for c in range(NCORES):
        xc = X[:, :, :, :, c * BC:(c + 1) * BC]    # [T, KD, 128, 2, BC]
        xc = np.ascontiguousarray(
            xc.reshape(T, KD, 128, 2, 2, HB).transpose(0, 1, 4, 2, 3, 5)
        ).reshape(T, KD, 2, 128, 2 * HB)           # [T, KD, half, 128, 2*HB]
        in_maps.append({"x": xc, "w1": w1p, "w2": w2p, "w3": w3p, "bias": bc,
                        "brow": br, "cw": cwh})

    trace = bool(os.environ.get("SNN_TRACE"))
    last_results = run_bass_kernel_spmd(nc, in_maps, core_ids=list(range(NCORES)),
                                        trace=trace)
    outp = np.empty((B, O), dtype=np.float32)
    for c in range(NCORES):
        outp[c * BC:(c + 1) * BC, :] = last_results.results[c]["out"].T
    return outp


# revision 33
# speedup vs baseline: 1.0136x; 1.0136x over previous
"""Trainium2 Bass kernel for NeuroVPR Vanilla SNN (3-layer LIF, T=3).

Data-parallel over batch: B=16384 -> 2048 per core x 8 cores.

Math (per timestep, per layer): v = (v_prev + h)/2; s = (v>=1); v *= (1-s).
The LIF recurrence is homogeneous and the decay is a power of 2, so each
layer runs in a scaled basis u_t = 2^t * 2c * v_t (c = weight prescale,
32 for L1 / 16 for L2-L3, lifting weights out of fp8's subnormal range):
    u_t = m_{t-1} + 2^t * psum_t      (the *0.5 decay cancels)
    s_t = (u_t >= 2^t * 2c)
    m_t = u_t * (u_t < 2^t * 2c)
The 2^t factor rides the ScalarE extract's free `scale` field; thresholds
double each timestep (exact powers of 2). Spike decisions match the fp32
recurrence up to matmul quantization error.

All matmuls are fp8e4 perf_mode=DoubleRow (K=256/instr, warm issue rate
216 ns at N=512 - measured 2x over bf16). Hidden-layer spikes live in a
+/-1 (ScalarE Sign, t=1) or +/-0.5 (VectorE is_ge/sub, t=0 and t=2)
basis: the next layer's ScalarE extract scale absorbs the basis change
(SSC) and the row-sum correction rides that layer's bias column, both
precomputed on host from the quantized weights. L1's bias rides a ones
row appended to x (pad row D) so its beta columns are zero.

Engine split per [128,1024] psum span (2 banks, measured costs):
  ScalarE: hb = Ident(SSC*psum + beta_col) -> bf16   (~1.09us)
  VectorE: u = m + hb          (tt add bf16, ~0.69us; t=0: u = hb)
  spike:   ScalarE Sign at t=1 (DVE-heavy step), VectorE ts otherwise
  VectorE: m = (u < th)*u      (stt bf16, ~1.22us; skipped at t=T-1)
At t=T-1, L1/L3 chains skip the extract and run on VectorE straight from
PSUM (u = dmsc*m + psum) so ScalarE never serializes the endgame; L3's
row-sum correction is then added into PSUM by a K=1 ones-matmul. All
t=T-1 chains process per-512 chunks (tile deps are range-based, so each
downstream matmul starts on the first half-span) and the final output
DMAs alternate sync/scalar queues.
GpSimd is avoided entirely (measured ~8us per [128,512] op).

Schedule: per timestep L1 runs as two half-batch passes (2 double-bank
psum groups each, k inner). L2(t-1)/L3(t-1) matmul groups are hooked
into the k-loops of later passes so the in-order PE queue never waits on
their PSUM-extract dependencies; only L2(T-1)+L3(T-1) trail the last
pass. x half-tiles ([128, 2048] fp8 per k-slab per half-batch) are
DMA-prefetched one L1 pass ahead.
"""
import os
import numpy as np
import ml_dtypes

B, T, D = 16384, 3, 2752
DP = 2816          # D padded to 11*256
KD = DP // 256     # 11 DoubleRow contraction slabs
H, O = 256, 100
OP = 112           # O padded so the DoubleRow pair-stride is 16B-aligned
NCORES = 8
BC = B // NCORES   # 2048
NB = 512           # matmul free-dim block
WB = 1024          # LIF elementwise span (2 psum banks)
HB = BC // 2       # half-batch per L1 pass (1024)

SC1, SC2 = 32.0, 8.0    # weight prescale: L1; L2/L3 (+/-1 spike basis)
TH1, TH2 = 64.0, 32.0   # base thresholds (scaled x2 each timestep)
EPS = 0.0625            # tie-break so Sign(u - (th-EPS)) == +/-1 with s=1 at u==th

_compiled = None
last_results = None  # BassKernelResults of the most recent run (for profiling)


def _build():
    from contextlib import ExitStack
    import concourse.bass as bass
    import concourse.mybir as mybir
    import concourse.tile as tile
    from concourse import bacc

    f8 = mybir.dt.float8e4
    bf16 = mybir.dt.bfloat16
    f32 = mybir.dt.float32
    A = mybir.AluOpType
    DR = mybir.MatmulPerfMode.DoubleRow
    IDENT = mybir.ActivationFunctionType.Identity
    SIGN = mybir.ActivationFunctionType.Sign

    nc = bacc.Bacc("TRN2", target_bir_lowering=False, debug=False)
    x = nc.dram_tensor("x", [T, KD, 2, 128, HB * 2], f8, kind="ExternalInput").ap()
    w1 = nc.dram_tensor("w1", [128, KD * 2 * H], f8, kind="ExternalInput").ap()
    w2 = nc.dram_tensor("w2", [128, 2 * H], f8, kind="ExternalInput").ap()
    w3 = nc.dram_tensor("w3", [128, 2 * OP], f8, kind="ExternalInput").ap()
    bias = nc.dram_tensor("bias", [128, 21], f32, kind="ExternalInput").ap()
    brow = nc.dram_tensor("brow", [1, 512], mybir.dt.bfloat16,
                          kind="ExternalInput").ap()
    out = nc.dram_tensor("out", [O, BC], f32, kind="ExternalOutput").ap()

    with tile.TileContext(nc) as tc, ExitStack() as ctx:
        wp = ctx.enter_context(tc.tile_pool(name="wp", bufs=1))
        xp = ctx.enter_context(tc.tile_pool(name="xp", bufs=24))
        pp1 = ctx.enter_context(tc.tile_pool(name="pp1", bufs=3, space="PSUM"))
        pp23 = ctx.enter_context(tc.tile_pool(name="pp23", bufs=1, space="PSUM"))
        sp = ctx.enter_context(tc.tile_pool(name="sp", bufs=1))
        tp = ctx.enter_context(tc.tile_pool(name="tp", bufs=6))

        # ---- weights / bias loads (k=0 slab first, on its own queue) ----
        w1t = wp.tile([128, KD * 2 * H], f8)
        nc.gpsimd.dma_start(out=w1t[:, 0:512], in_=w1[:, 0:512])
        for c0, c1 in ((512, 2048), (2048, 4096), (4096, KD * 512)):
            nc.scalar.dma_start(out=w1t[:, c0:c1], in_=w1[:, c0:c1])
        w1o = w1t[:, :].rearrange("p (k two m) -> p k two m", k=KD, two=2)
        w2t = wp.tile([128, 2 * H], f8)
        nc.gpsimd.dma_start(out=w2t[:, :], in_=w2[:, :])
        w2o = w2t[:, :].rearrange("p (two m) -> p two m", two=2)
        w3t = wp.tile([128, 2 * OP], f8)
        nc.gpsimd.dma_start(out=w3t[:, :], in_=w3[:, :])
        w3o = w3t[:, :].rearrange("p (two m) -> p two m", two=2)
        bt = wp.tile([128, 21], f32)
        nc.gpsimd.dma_start(out=bt[:, :], in_=bias[:, :])
        browt = wp.tile([1, 512], bf16)
        nc.gpsimd.dma_start(out=browt[:, :], in_=brow[:, :])
        ones = wp.tile([1, NB], bf16)
        nc.vector.memset(ones[:, :], 1.0)
        # column layout (host fills): 0-5 beta1[t,h]; 6-11 beta2[t,h];
        # 12-14 beta3[t]; 15-17 -(2^t*TH1-EPS); 18-20 -(2^t*TH2-EPS)
        B1 = lambda t, h: bt[:, 2 * t + h: 2 * t + h + 1]
        B2 = lambda t, h: bt[:, 6 + 2 * t + h: 6 + 2 * t + h + 1]
        B3 = lambda t: bt[:, 12 + t: 13 + t]
        N1 = lambda t: bt[:, 15 + t: 16 + t]
        N2 = lambda t: bt[:, 18 + t: 19 + t]

        # ACT warmup: pull the activation table load off the critical path
        wu = wp.tile([128, 8], bf16)
        nc.vector.memset(wu[:, :], 0.0)
        nc.scalar.activation(wu[:, 0:4], wu[:, 4:8], IDENT, bias=bt[:, 0:1])

        # ---- persistent state (m = scaled membrane, written at t=0) ----
        m1 = [sp.tile([128, BC], bf16, tag=f"m1_{h}", name=f"m1_{h}")
              for h in range(2)]
        m2 = [sp.tile([128, BC], bf16, tag=f"m2_{h}", name=f"m2_{h}")
              for h in range(2)]
        m3 = sp.tile([128, BC], bf16, tag="m3")
        s1 = sp.tile([128, 2 * BC], f8, tag="s1")
        s2 = sp.tile([128, 2 * BC], f8, tag="s2")
        s1r = s1[:, :].rearrange("p (two n) -> p two n", two=2)
        s2r = s2[:, :].rearrange("p (two n) -> p two n", two=2)
        outsb = sp.tile([128, BC], f32, tag="outsb")

        xt = {}  # (t, k, half) -> x tile handle [128, 2*HB]

        def x_fetch(t, k, half, split=False):
            xt[t, k, half] = xp.tile([128, 2 * HB], f8, tag="x",
                                     name=f"x_{t}_{k}_{half}")
            if split:
                nc.sync.dma_start(out=xt[t, k, half][:, 0:HB],
                                  in_=x[t, k, half, :, 0:HB])
                nc.gpsimd.dma_start(out=xt[t, k, half][:, HB:],
                                    in_=x[t, k, half, :, HB:])
            else:
                nc.sync.dma_start(out=xt[t, k, half][:, :],
                                  in_=x[t, k, half, :, :])

        SSC = [2.0, 2.0, 8.0]  # 2^t x (2 if spikes were +/-0.5 basis)

        def lif(ps, m_ap, s_ap, bcol, nthcol, th, t, P=128, out_f32=False,
                sc=None, dmsc=0.25, dth=None, pd=True):
            """Scaled-LIF on one [P, WB] psum span. At t == T-1 the whole
            chain runs on VectorE straight from PSUM (dmsc*m + psum, spike at
            dth) so ScalarE never serializes the endgame; earlier timesteps
            use the ScalarE extract + bf16 VectorE ops."""
            if t == T - 1 and pd:
                u = tp.tile([128, WB], bf16, tag="u", name="u")[:P, :]
                for c in range(2):
                    cs = slice(c * NB, (c + 1) * NB)
                    nc.vector.scalar_tensor_tensor(u[:, cs], m_ap[:, cs], dmsc,
                                                   ps[:, cs], A.mult, A.add)
                    if out_f32:
                        nc.vector.tensor_scalar(s_ap[:, cs], u[:, cs], dth,
                                                None, A.is_ge)
                    else:
                        nc.vector.tensor_scalar(s_ap[:, cs], u[:, cs], dth,
                                                0.5, A.is_ge, A.subtract)
                return
            if t == T - 1:
                # PATH-A endgame (L2): per-512 chunks so downstream range
                # consumers start on the first half
                hb = tp.tile([128, WB], bf16, tag="hb", name="hb")[:P, :]
                u = tp.tile([128, WB], bf16, tag="u", name="u")[:P, :]
                for c in range(2):
                    cs = slice(c * NB, (c + 1) * NB)
                    nc.scalar.activation(hb[:, cs], ps[:, cs], IDENT,
                                         bias=bcol[:P, :], scale=sc)
                    nc.vector.tensor_tensor(u[:, cs], m_ap[:, cs], hb[:, cs],
                                            A.add)
                    nc.vector.tensor_scalar(s_ap[:, cs], u[:, cs],
                                            th * 2 ** t, 0.5,
                                            A.is_ge, A.subtract)
                return
            hb = tp.tile([128, WB], bf16, tag="hb", name="hb")[:P, :]
            nc.scalar.activation(hb, ps, IDENT, bias=bcol[:P, :],
                                 scale=float(2 ** t) if sc is None else sc)
            if t == 0:
                u = hb
            else:
                u = tp.tile([128, WB], bf16, tag="u", name="u")[:P, :]
                nc.vector.tensor_tensor(u, m_ap, hb, A.add)
            if s_ap is not None:
                if t == 1:
                    nc.scalar.activation(s_ap, u, SIGN, bias=nthcol[:P, :])
                else:
                    nc.vector.tensor_scalar(s_ap, u, th * 2 ** t, 0.5,
                                            A.is_ge, A.subtract)
            nc.vector.scalar_tensor_tensor(m_ap, u, th * 2 ** t, u,
                                           A.is_lt, A.mult)

        def l1_pass(t, half, hooks=None):
            """One half-batch L1 pass: 2 double-bank psum groups, k inner.
            Prefetches the next pass's x tiles; `hooks[k]` emits interleaved
            L2/L3 work (their MMs slot into the PE stream between slabs)."""
            boff = half * HB
            ps = [pp1.tile([128, WB], f32, tag="ps1", name=f"ps1_{t}_{half}_{h}")
                  for h in range(2)]
            for k in range(KD):
                for fn in (hooks or {}).get(k, []):
                    fn()
                xr = xt[t, k, half][:, :].rearrange("p (two n) -> p two n", two=2)
                for h in range(2):
                    for b in range(2):
                        nc.tensor.matmul(
                            ps[h][:, b * NB:(b + 1) * NB],
                            w1o[:, k, :, h * 128:(h + 1) * 128],
                            xr[:, :, b * NB:(b + 1) * NB],
                            start=(k == 0), stop=(k == KD - 1), perf_mode=DR,
                            skip_group_check=True)
                if half == 0:
                    x_fetch(t, k, 1)
                elif t < T - 1:
                    x_fetch(t + 1, k, 0)
            for h in range(2):
                bs = slice(boff, boff + WB)
                lif(ps[h][:, :], m1[h][:, bs],
                    s1[:, h * BC + boff: h * BC + boff + WB],
                    B1(t, h), N1(t), TH1, t, dmsc=0.25, dth=TH1)

        def l2_group(t, h, bp, pool, tag):
            ps2 = pool.tile([128, WB], f32, tag=tag, name=f"ps2_{t}_{h}_{bp}")
            for b in range(2):
                nc.tensor.matmul(ps2[:, b * NB:(b + 1) * NB],
                                 w2o[:, :, h * 128:(h + 1) * 128],
                                 s1r[:, :, (2 * bp + b) * NB:(2 * bp + b + 1) * NB],
                                 start=True, stop=True, perf_mode=DR,
                                 skip_group_check=True)
            return ps2

        def l3_group(t, bp, pool, tag):
            ps3 = pool.tile([128, WB], f32, tag=tag, name=f"ps3_{t}_{bp}")
            corr = (t == T - 1)
            for b in range(2):
                nc.tensor.matmul(ps3[:OP, b * NB:(b + 1) * NB], w3o[:, :, :],
                                 s2r[:, :, (2 * bp + b) * NB:(2 * bp + b + 1) * NB],
                                 start=True, stop=not corr, perf_mode=DR,
                                 skip_group_check=True)
                if corr:
                    nc.tensor.matmul(ps3[:OP, b * NB:(b + 1) * NB],
                                     browt[0:1, 256:256 + OP],
                                     ones[0:1, :], start=False, stop=True,
                                     skip_group_check=True)
            return ps3

        def l2_one(t, h, bp, pool=None, tag=None):
            bs = slice(bp * WB, (bp + 1) * WB)
            ps2 = l2_group(t, h, bp, pool or pp23, tag or "ps23")
            lif(ps2[:, :], m2[h][:, bs],
                s2[:, h * BC + bp * WB: h * BC + (bp + 1) * WB],
                B2(t, h), N2(t), TH2, t, sc=SSC[t], pd=False)

        def l3_one(t, bp, pool=None, tag=None):
            bs = slice(bp * WB, (bp + 1) * WB)
            ps3 = l3_group(t, bp, pool or pp23, tag or "ps23")
            if t != T - 1:
                lif(ps3[:OP, :], m3[:OP, bs], None, B3(t), None, TH2, t,
                    P=OP, sc=SSC[t])
            else:
                lif(ps3[:OP, :], m3[:OP, bs], outsb[:OP, bs], B3(t), None,
                    TH2, t, P=OP, out_f32=True, sc=SSC[t], dmsc=0.125,
                    dth=16.0)
                for c in range(2):
                    cs = slice(bp * WB + c * NB, bp * WB + (c + 1) * NB)
                    (nc.sync if c == 0 else nc.scalar).dma_start(
                        out=out[:, cs], in_=outsb[:O, cs])

        for k in range(KD):
            x_fetch(0, k, 0)
        l1_pass(0, 0)
        l1_pass(0, 1)
        l1_pass(1, 0)
        l1_pass(1, 1, hooks={3: [lambda: l2_one(0, 0, 0)],
                             5: [lambda: l2_one(0, 0, 1)],
                             7: [lambda: l2_one(0, 1, 0)],
                             9: [lambda: l2_one(0, 1, 1)]})
        l1_pass(2, 0, hooks={1: [lambda: l2_one(1, 0, 0)],
                             3: [lambda: l2_one(1, 0, 1)],
                             5: [lambda: l2_one(1, 1, 0)],
                             7: [lambda: l2_one(1, 1, 1)],
                             9: [lambda: l3_one(0, 0)]})
        l1_pass(2, 1, hooks={2: [lambda: l3_one(0, 1)],
                             6: [lambda: l3_one(1, 0)],
                             9: [lambda: l3_one(1, 1)]})
        # endgame: only L2(T-1) (ScalarE extract) + L3(T-1) (VectorE direct)
        t_ = T - 1
        l2_one(t_, 0, 0, pp1, "ps1")
        l2_one(t_, 1, 0, pp1, "ps1")
        l2_one(t_, 0, 1, pp1, "ps1")
        l2_one(t_, 1, 1, pp1, "ps1")
        l3_one(t_, 0)
        l3_one(t_, 1)

    nc.compile()
    return nc


def kernel(dvs, W1, b1, W2, b2, W3, b3):
    global _compiled, last_results
    from concourse.bass_utils import run_bass_kernel_spmd

    if _compiled is None:
        _compiled = _build()
    nc = _compiled

    f8 = ml_dtypes.float8_e4m3

    def q8(a, scale):
        return np.clip(a * scale, -240.0, 240.0).astype(f8)

    # x: [B, T, D] -> fp8 [T, KD, 128, 2, B]  (d = k*256 + two*128 + p)
    x8 = q8(dvs, 1.0).transpose(1, 2, 0)          # [T, D, B]
    X = np.zeros((T, KD, 2, 128, B), dtype=f8)
    X.reshape(T, DP, B)[:, :D, :] = x8
    X.reshape(T, DP, B)[:, D, :] = f8(1.0)        # bias row (w1 row D = c1*b1)
    X = np.ascontiguousarray(X.transpose(0, 1, 3, 2, 4))  # [T, KD, 128, 2, B]

    # w1: [DP, H] scaled by SC1 -> [128, KD, 2, H]
    w1p = np.zeros((KD, 2, 128, H), dtype=f8)
    w1p.reshape(DP, H)[:D, :] = q8(W1.T, SC1)
    w1p.reshape(DP, H)[D, :] = q8(b1, SC1)
    w1p = np.ascontiguousarray(w1p.transpose(2, 0, 1, 3)).reshape(128, KD * 2 * H)
    # w2/w3 scaled by SC2 (+/-1 spike basis)
    w2q = q8(W2.T, SC2)                            # [H, H] j-major
    w2p = np.ascontiguousarray(
        w2q.reshape(2, 128, H).transpose(1, 0, 2)).reshape(128, 2 * H)
    w3q = np.zeros((H, OP), dtype=f8)
    w3q[:, :O] = q8(W3.T, SC2)
    w3p = np.ascontiguousarray(
        w3q.reshape(2, 128, OP).transpose(1, 0, 2)).reshape(128, 2 * OP)

    # bias/threshold columns; row-sum corrections use the quantized weights
    rs2 = w2q.astype(np.float64).sum(axis=0)       # [H]
    rs3 = w3q.astype(np.float64).sum(axis=0)       # [OP]
    bc = np.zeros((128, 21), dtype=np.float32)
    for t in range(T):
        p2 = float(2 ** t)
        for h in range(2):
            bc[:, 6 + 2 * t + h] = p2 * (rs2[h * 128:(h + 1) * 128]
                                         + 2 * SC2 * b2[h * 128:(h + 1) * 128])
        bc[:OP, 12 + t] = p2 * rs3
        bc[:O, 12 + t] += p2 * 2 * SC2 * b3
        bc[:, 15 + t] = -(p2 * TH1 - EPS)
        bc[:, 18 + t] = -(p2 * TH2 - EPS)

    br = np.zeros((1, 512), dtype=np.float32)
    br[0, :H] = (rs2 + 2 * SC2 * b2) / 2
    br[0, H:H + OP] = rs3 / 2
    br[0, H:H + O] += SC2 * b3
    br = br.astype(ml_dtypes.bfloat16)

    in_maps = []
    for c in range(NCORES):
        xc = X[:, :, :, :, c * BC:(c + 1) * BC]    # [T, KD, 128, 2, BC]
        xc = np.ascontiguousarray(
            xc.reshape(T, KD, 128, 2, 2, HB).transpose(0, 1, 4, 2, 3, 5)
        ).reshape(T, KD, 2, 128, 2 * HB)           # [T, KD, half, 128, 2*HB]
        in_maps.append({"x": xc, "w1": w1p, "w2": w2p, "w3": w3p, "bias": bc,
                        "brow": br})

    trace = bool(os.environ.get("SNN_TRACE"))
    last_results = run_bass_kernel_spmd(nc, in_maps, core_ids=list(range(NCORES)),
                                        trace=trace)
    outp = np.empty((B, O), dtype=np.float32)
    for c in range(NCORES):
        outp[c * BC:(c + 1) * BC, :] = last_results.results[c]["out"].T
    return outp
